# revision 34
# baseline (speedup 1.0000x reference)
"""AnyPrecisionLinear (4-bit LUT quantized linear) on 8 Trainium2 NeuronCores.

y[b,t,o] = sum_i x[b,t,i] * lut[o, idx(o,i)] + bias[o]
  idx(o,i) = 4-bit code assembled LSB-first from bit-planes qweight[0:4].

Sharding (column-parallel): qweight/lut/bias sharded along out_features into
8 shards of 512; x replicated; each core computes y[:, :, shard] and the host
concatenates along the feature axis.

Per-core kernel — transposed-output GEMM + host bit-unpack:
 - Host pre-expands the qweight bit-planes into a bf16 plane-0 tensor and
   int16 plane-1..3 masks (pure bit-unpacking / dtype layout transform; all
   lut-dependent arithmetic stays on device).  This removes the int16 AND
   chain from the DVE and makes dequant a clean 2-stage ACT->DVE pipeline:
   8 ACT affine candidates from plane 0, 3 predicated folds by planes 1..3.
 - Computes yT[o, t]: stationary = dequanted WT block [i, 128o], moving =
   x strip [i, 512 tokens]; every matmul is per-output-block AND 512-wide so
   the PE consumes each 128-channel block the moment it is dequanted.
   The host un-transposes the [O, T] result for free.
 - Schedule: NAG staged 512-token groups run one PSUM pass per dequant chunk
   (bf16 partials in SBUF); pass 0 runs as two output-block-pair sweeps with
   independent x tiles so early groups don't hold x hostage while later
   blocks dequant.  Remaining groups run fully fused (128 matmuls into 4
   resident PSUM banks, no staging).
 - bias is a per-partition ACT bias fused into the PSUM-evacuation copy.
 - x fed as bf16 (halves HBM read traffic); all DMA on HWDGE queues.
"""
import numpy as np
import ml_dtypes
from concourse import bacc, mybir, tile, masks
from concourse.bass_utils import run_bass_kernel_spmd

dt = mybir.dt
F32, BF16, I32, I16 = dt.float32, dt.bfloat16, dt.int32, dt.int16
Act = mybir.ActivationFunctionType
Alu = mybir.AluOpType

N_CORES = 8
B, TT, IN, OF = 4, 2048, 4096, 4096
T = B * TT
O = OF // N_CORES
TG = 512                         # tokens per group
# Note: DMA-XBAR weight transpose was tried and measured 1.23us per
# [128,128] block on HW (10x the cost model); PE transposes win.


def _build(T=8192, IN=4096, O=512, NAG=8):
    n_g = T // TG                # 16 token groups
    n_it = IN // 128             # 32 i-tiles
    n_ob = O // 128              # 4 output blocks
    n_ch = 4                     # dequant chunks
    CHI = IN // n_ch             # 1024 input features per chunk
    n_cit = CHI // 128           # 8 i-tiles per chunk
    NH = n_cit // 2              # 4 i-tiles per half-chunk x tile

    nc = bacc.Bacc("TRN2", target_bir_lowering=False, debug=False)
    x_d = nc.dram_tensor("x", [n_g, 128, n_it, TG], BF16, kind="ExternalInput")
    b0_d = nc.dram_tensor("b0", [O, IN], BF16, kind="ExternalInput")
    m1_d = nc.dram_tensor("m1", [O, IN], I16, kind="ExternalInput")
    m2_d = nc.dram_tensor("m2", [O, IN], I16, kind="ExternalInput")
    m3_d = nc.dram_tensor("m3", [O, IN], I16, kind="ExternalInput")
    lut_d = nc.dram_tensor("lut", [O, 16], F32, kind="ExternalInput")
    biasT_d = nc.dram_tensor("biasT", [128, n_ob], F32, kind="ExternalInput")
    out_d = nc.dram_tensor("out", [O, T], F32, kind="ExternalOutput")

    with tile.TileContext(nc) as tc:
        with tc.tile_pool(name="consts", bufs=1) as consts, \
             tc.tile_pool(name="wpool", bufs=1) as wpool, \
             tc.tile_pool(name="qpool", bufs=1) as qpool, \
             tc.tile_pool(name="dq", bufs=2) as dq, \
             tc.tile_pool(name="wd", bufs=7) as wd, \
             tc.tile_pool(name="xq", bufs=12) as xq, \
             tc.tile_pool(name="ysb", bufs=NAG * 4 + 16) as ysb, \
             tc.tile_pool(name="yout", bufs=4) as yout, \
             tc.tile_pool(name="psw", bufs=2, space="PSUM") as psw, \
             tc.tile_pool(name="psg", bufs=6, space="PSUM") as psg:

            ident_bf16 = consts.tile([128, 128], BF16)
            masks.make_identity(nc, ident_bf16[:])
            biasT = consts.tile([128, n_ob], F32)

            WT = wpool.tile([128, n_it, O], BF16)  # [i-part, i-tile, o]

            cks, dks = [], []

            def load_lut(ob):
                lut_t = qpool.tile([128, 16], F32, name=f"lut_t{ob}", tag=f"lut_t{ob}")
                nc.sync.dma_start(lut_t[:], lut_d[ob*128:(ob+1)*128, :])
                ck = qpool.tile([128, 8], F32, name=f"ck{ob}", tag=f"ck{ob}")
                dk = qpool.tile([128, 8], F32, name=f"dk{ob}", tag=f"dk{ob}")
                nc.vector.tensor_copy(ck[:], lut_t[:, 0:16:2])
                nc.vector.tensor_tensor(dk[:], lut_t[:, 1:16:2],
                                        lut_t[:, 0:16:2], Alu.subtract)
                cks.append(ck); dks.append(dk)

            wds = {}

            def _transpose_wt(w_ap, ch, ob, j0=0, nj=None):
                nj = n_cit if nj is None else nj
                it0 = ch * n_cit + j0
                wt_ps = psw.tile([128, n_cit * 128], BF16, name="wt_ps", tag="wt_ps")
                for j in range(nj):
                    nc.tensor.transpose(wt_ps[:, j*128:(j+1)*128],
                                        w_ap[:, j*128:(j+1)*128], ident_bf16[:])
                nc.scalar.copy(WT[:, it0:it0+nj, ob*128:(ob+1)*128],
                               wt_ps[:, :nj*128].rearrange("p (a b) -> p a b", a=nj))

            def dq_unit(ch, ob, j0, nj, direct=False, vsplit=False):
                """Candidates (ACT/DVE/gpsimd) + folds (DVE) for nj i-tiles."""
                ck, dk = cks[ob], dks[ob]
                CU = nj * 128
                osl = slice(ob * 128, (ob + 1) * 128)
                isl = slice(ch * CHI + j0 * 128, ch * CHI + j0 * 128 + CU)
                b0 = dq.tile([128, CHI], BF16, name="b0", tag="b0")
                m1 = dq.tile([128, CHI], I16, name="m1", tag="m1")
                m2 = dq.tile([128, CHI], I16, name="m2", tag="m2")
                m3 = dq.tile([128, CHI], I16, name="m3", tag="m3")
                nc.sync.dma_start(b0[:, :CU], b0_d[osl, isl])
                nc.sync.dma_start(m1[:, :CU], m1_d[osl, isl])
                nc.sync.dma_start(m2[:, :CU], m2_d[osl, isl])
                nc.sync.dma_start(m3[:, :CU], m3_d[osl, isl])
                V = dq.tile([128, 8, CHI], BF16, name="V", tag="V")
                for k in range(8):
                    if vsplit and k >= 5:
                        nc.vector.tensor_scalar(V[:, k, :CU], b0[:, :CU],
                                                dk[:, k:k+1], ck[:, k:k+1],
                                                Alu.mult, Alu.add)
                    elif k >= 6 or (vsplit and k >= 3):
                        nc.gpsimd.tensor_scalar(V[:, k, :CU], b0[:, :CU],
                                                dk[:, k:k+1], ck[:, k:k+1],
                                                Alu.mult, Alu.add)
                    else:
                        nc.scalar.activation(V[:, k, :CU], b0[:, :CU], Act.Identity,
                                             bias=ck[:, k:k+1], scale=dk[:, k:k+1])
                nc.vector.copy_predicated(
                    V[:, 0:8:2, :CU], m1[:, :CU].unsqueeze(1).broadcast_to([128, 4, CU]),
                    V[:, 1:8:2, :CU])
                nc.vector.copy_predicated(
                    V[:, 0:8:4, :CU], m2[:, :CU].unsqueeze(1).broadcast_to([128, 2, CU]),
                    V[:, 2:8:4, :CU])
                if direct:
                    nc.vector.copy_predicated(V[:, 0, :CU], m3[:, :CU], V[:, 4, :CU])
                    _transpose_wt(V[:, 0, :CU], ch, ob, j0, nj)
                else:
                    w_t = wd.tile([128, CHI], BF16, name="wd", tag="wd")
                    nc.vector.select(w_t[:, :CU], m3[:, :CU], V[:, 4, :CU], V[:, 0, :CU])
                    wds[(ch, ob)] = w_t

            def dq_front(ch, ob, direct=False, vsplit=False):
                dq_unit(ch, ob, 0, n_cit, direct=direct, vsplit=vsplit)

            def dq_back(ch, ob):
                _transpose_wt(wds.pop((ch, ob))[:], ch, ob)

            y_sbs = {}

            def _load_half(g, ch, h):
                xt = xq.tile([128, NH, TG], BF16, name="xt", tag="xt")
                it0 = ch * n_cit + h * NH
                for q in range(2):
                    nc.sync.dma_start(xt[:, q*(NH//2):(q+1)*(NH//2), :],
                                      x_d[g][:, it0 + q*(NH//2):it0 + (q+1)*(NH//2), :])
                return xt

            def emit_pass(ch, g, obs):
                first, last = (ch == 0), (ch == n_ch - 1)
                it0 = ch * n_cit
                xts = [_load_half(g, ch, h) for h in range(2)]
                for ob in obs:
                    osl = slice(ob * 128, (ob + 1) * 128)
                    y_ps = psg.tile([128, TG], F32, name="y_ps", tag="y_ps")
                    for j in range(n_cit):
                        nc.tensor.matmul(y_ps[:], WT[:, it0 + j, osl],
                                         xts[j // NH][:, j % NH, :],
                                         start=(j == 0), stop=(j == n_cit - 1))
                    if first:
                        y_sb = ysb.tile([128, TG], BF16, name="y_sb", tag="y_sb")
                        nc.scalar.activation(y_sb[:], y_ps[:], Act.Identity,
                                             bias=biasT[:, ob:ob+1])
                        y_sbs[(g, ob)] = y_sb
                    elif not last:
                        y_sb2 = ysb.tile([128, TG], BF16, name="y_sb", tag="y_sb")
                        nc.vector.tensor_tensor(y_sb2[:], y_ps[:],
                                                y_sbs[(g, ob)][:], Alu.add)
                        y_sbs[(g, ob)] = y_sb2
                    else:
                        y_o = yout.tile([128, TG], F32, name="y_o", tag="y_o")
                        nc.vector.tensor_tensor(y_o[:], y_ps[:],
                                                y_sbs.pop((g, ob))[:], Alu.add)
                        nc.sync.dma_start(out_d[ob*128:(ob+1)*128, g*TG:(g+1)*TG],
                                          y_o[:])

            def emit_fused(g):
                y_pss = [psg.tile([128, TG], F32, name="y_ps", tag="y_ps")
                         for _ in range(n_ob)]
                for q in range(n_it // NH):       # 8 half-chunks
                    xt = _load_half(g, q // 2, q % 2)
                    for ob in range(n_ob):
                        osl = slice(ob * 128, (ob + 1) * 128)
                        for j in range(NH):
                            nc.tensor.matmul(
                                y_pss[ob][:], WT[:, q*NH + j, osl], xt[:, j, :],
                                start=(q == 0 and j == 0),
                                stop=(q == n_it // NH - 1 and j == NH - 1))
                for ob in range(n_ob):
                    y_o = yout.tile([128, TG], F32, name="y_o", tag="y_o")
                    nc.scalar.activation(y_o[:], y_pss[ob][:], Act.Identity,
                                         bias=biasT[:, ob:ob+1])
                    if g == n_g - 1:  # shorter tail: parallel half-DMAs
                        for h in range(2):
                            ts = slice(g*TG + h*(TG//2), g*TG + (h+1)*(TG//2))
                            nc.sync.dma_start(out_d[ob*128:(ob+1)*128, ts],
                                              y_o[:, h*(TG//2):(h+1)*(TG//2)])
                    else:
                        nc.sync.dma_start(out_d[ob*128:(ob+1)*128, g*TG:(g+1)*TG],
                                          y_o[:])

            # ---- schedule ----
            # Chunk 0 dequants in half-units so the first weight tiles land
            # ~13us earlier; the pass-0 matmuls for the first half start
            # before the second half's folds finish (region-tracked deps).
            for ob in range(n_ob):
                load_lut(ob)
                if ob == 0:
                    for h in range(2):
                        dq_unit(0, 0, h * NH, NH, direct=True, vsplit=True)
                else:
                    dq_front(0, ob, direct=True)
            nc.sync.dma_start(biasT[:], biasT_d[:])

            # Pass 0 as two ob-pair sweeps (own x tiles -> no x hostage while
            # later blocks dequant); chunk-1 fronts interleaved.
            for si, obs in enumerate(((0, 1), (2, 3))):
                for g in range(NAG):
                    if g < 2:
                        dq_front(1, si * 2 + g)
                    emit_pass(0, g, obs)

            for ch in range(1, n_ch):
                for ob in range(n_ob):
                    dq_back(ch, ob)
                for g in range(NAG):
                    if ch + 1 < n_ch and g < n_ob:
                        dq_front(ch + 1, g)
                    emit_pass(ch, g, range(n_ob))

            for g in range(NAG, n_g):
                emit_fused(g)

    nc.compile()
    return nc


def _make_xt(x2):
    # X6[g, p, a, t] = x2[g*TG+t, a*128+p] in bf16 — pure layout change
    xb = x2.astype(ml_dtypes.bfloat16)
    return np.ascontiguousarray(
        xb.reshape(T // TG, TG, IN // 128, 128).transpose(0, 3, 2, 1))


def _make_inmaps(x, qweight, lut, bias):
    x6 = _make_xt(np.asarray(x, np.float32).reshape(T, IN))
    # Bit-plane unpack (layout/dtype transform only; no lut arithmetic):
    # plane-0 bits as bf16 0/1, planes 1..3 as int16 0/1 fold predicates.
    qw = np.asarray(qweight, np.int32)[:4]
    shifts = np.arange(32, dtype=np.int32)
    bits = ((qw[:, :, :, None] >> shifts) & 1).astype(np.int16).reshape(4, OF, IN)
    b0f = bits[0].astype(ml_dtypes.bfloat16)
    in_maps = []
    for c in range(N_CORES):
        sl = slice(c * O, (c + 1) * O)
        bsl = np.asarray(bias, np.float32)[sl]
        in_maps.append({
            "x": x6,
            "b0": np.ascontiguousarray(b0f[sl]),
            "m1": np.ascontiguousarray(bits[1, sl]),
            "m2": np.ascontiguousarray(bits[2, sl]),
            "m3": np.ascontiguousarray(bits[3, sl]),
            "lut": np.ascontiguousarray(np.asarray(lut, np.float32)[sl, :]),
            "biasT": np.ascontiguousarray(bsl.reshape(O // 128, 128).T),
        })
    return in_maps


_nc_cache = None


def kernel(x, qweight, lut, bias, w_bits):
    global _nc_cache
    assert int(w_bits) == 4, f"kernel hardcodes w_bits=4, got {w_bits}"
    x = np.asarray(x, dtype=np.float32)
    assert x.shape == (B, TT, IN) and np.asarray(qweight).shape[1:] == (OF, IN // 32)

    in_maps = _make_inmaps(x, qweight, lut, bias)
    if _nc_cache is None:
        _nc_cache = _build(T, IN, O)
    res = run_bass_kernel_spmd(_nc_cache, in_maps, core_ids=list(range(N_CORES)))
    y = np.concatenate([res.results[i]["out"].T for i in range(N_CORES)], axis=1)
    return np.ascontiguousarray(y.reshape(B, TT, OF).astype(np.float32))


# revision 36
# speedup vs baseline: 1.1638x; 1.1638x over previous
"""AnyPrecisionLinear (4-bit LUT quantized linear) on 8 Trainium2 NeuronCores.

y[b,t,o] = sum_i x[b,t,i] * lut[o, idx(o,i)] + bias[o]
  idx(o,i) = 4-bit code assembled LSB-first from bit-planes qweight[0:4].

Sharding (column-parallel): qweight/lut/bias sharded along out_features into
8 shards of 512; x replicated; each core computes y[:, :, shard] and the host
concatenates along the feature axis.

Per-core kernel — transposed-output GEMM + host bit-unpack:
 - Host pre-expands the qweight bit-planes into a bf16 plane-0 tensor and
   int16 plane-1..3 masks (pure bit-unpacking / dtype layout transform; all
   lut-dependent arithmetic stays on device).  This removes the int16 AND
   chain from the DVE and makes dequant a clean 2-stage ACT->DVE pipeline:
   8 ACT affine candidates from plane 0, 3 predicated folds by planes 1..3.
 - Computes yT[o, t]: stationary = dequanted WT block [i, 128o], moving =
   x strip [i, 512 tokens]; every matmul is per-output-block AND 512-wide so
   the PE consumes each 128-channel block the moment it is dequanted.
   The host un-transposes the [O, T] result for free.
 - Schedule: NAG staged 512-token groups run one PSUM pass per dequant chunk
   (bf16 partials in SBUF); pass 0 runs as two output-block-pair sweeps with
   independent x tiles so early groups don't hold x hostage while later
   blocks dequant.  Remaining groups run fully fused (128 matmuls into 4
   resident PSUM banks, no staging).
 - bias is a per-partition ACT bias fused into the PSUM-evacuation copy.
 - x fed as bf16 (halves HBM read traffic); all DMA on HWDGE queues.
"""
import numpy as np
import ml_dtypes
from concourse import bacc, mybir, tile, masks
from concourse.bass_utils import run_bass_kernel_spmd

dt = mybir.dt
F32, BF16, I32, I16 = dt.float32, dt.bfloat16, dt.int32, dt.int16
Act = mybir.ActivationFunctionType
Alu = mybir.AluOpType

N_CORES = 8
B, TT, IN, OF = 4, 2048, 4096, 4096
T = B * TT
O = OF // N_CORES
TG = 512                         # tokens per group
# Note: DMA-XBAR weight transpose was tried and measured 1.23us per
# [128,128] block on HW (10x the cost model); PE transposes win.


def _build(T=8192, IN=4096, O=512, NAG=8):
    n_g = T // TG                # 16 token groups
    n_it = IN // 128             # 32 i-tiles
    n_ob = O // 128              # 4 output blocks
    n_ch = 4                     # dequant chunks
    CHI = IN // n_ch             # 1024 input features per chunk
    n_cit = CHI // 128           # 8 i-tiles per chunk
    NH = n_cit // 2              # 4 i-tiles per half-chunk x tile

    nc = bacc.Bacc("TRN2", target_bir_lowering=False, debug=False)
    x_d = nc.dram_tensor("x", [n_g, 128, n_it, TG], BF16, kind="ExternalInput")
    b0_d = nc.dram_tensor("b0", [O, IN], BF16, kind="ExternalInput")
    m1_d = nc.dram_tensor("m1", [O, IN], I16, kind="ExternalInput")
    m2_d = nc.dram_tensor("m2", [O, IN], I16, kind="ExternalInput")
    m3_d = nc.dram_tensor("m3", [O, IN], I16, kind="ExternalInput")
    lut_d = nc.dram_tensor("lut", [O, 16], F32, kind="ExternalInput")
    biasT_d = nc.dram_tensor("biasT", [128, n_ob], F32, kind="ExternalInput")
    out_d = nc.dram_tensor("out", [O, T], F32, kind="ExternalOutput")

    with tile.TileContext(nc) as tc:
        with tc.tile_pool(name="consts", bufs=1) as consts, \
             tc.tile_pool(name="wpool", bufs=1) as wpool, \
             tc.tile_pool(name="qpool", bufs=1) as qpool, \
             tc.tile_pool(name="dq", bufs=2) as dq, \
             tc.tile_pool(name="wd", bufs=7) as wd, \
             tc.tile_pool(name="xq", bufs=12) as xq, \
             tc.tile_pool(name="ysb", bufs=NAG * 4 + 16) as ysb, \
             tc.tile_pool(name="yout", bufs=4) as yout, \
             tc.tile_pool(name="psw", bufs=2, space="PSUM") as psw, \
             tc.tile_pool(name="psg", bufs=6, space="PSUM") as psg:

            ident_bf16 = consts.tile([128, 128], BF16)
            masks.make_identity(nc, ident_bf16[:])
            biasT = consts.tile([128, n_ob], F32)

            WT = wpool.tile([128, n_it, O], BF16)  # [i-part, i-tile, o]

            cks, dks = [], []

            def load_lut(ob):
                lut_t = qpool.tile([128, 16], F32, name=f"lut_t{ob}", tag=f"lut_t{ob}")
                nc.sync.dma_start(lut_t[:], lut_d[ob*128:(ob+1)*128, :])
                ck = qpool.tile([128, 8], F32, name=f"ck{ob}", tag=f"ck{ob}")
                dk = qpool.tile([128, 8], F32, name=f"dk{ob}", tag=f"dk{ob}")
                nc.vector.tensor_copy(ck[:], lut_t[:, 0:16:2])
                nc.vector.tensor_tensor(dk[:], lut_t[:, 1:16:2],
                                        lut_t[:, 0:16:2], Alu.subtract)
                cks.append(ck); dks.append(dk)

            wds = {}

            def _transpose_wt(w_ap, ch, ob, j0=0, nj=None):
                nj = n_cit if nj is None else nj
                it0 = ch * n_cit + j0
                wt_ps = psw.tile([128, n_cit * 128], BF16, name="wt_ps", tag="wt_ps")
                for j in range(nj):
                    nc.tensor.transpose(wt_ps[:, j*128:(j+1)*128],
                                        w_ap[:, j*128:(j+1)*128], ident_bf16[:])
                nc.scalar.copy(WT[:, it0:it0+nj, ob*128:(ob+1)*128],
                               wt_ps[:, :nj*128].rearrange("p (a b) -> p a b", a=nj))

            def dq_unit(ch, ob, j0, nj, direct=False, vsplit=False):
                """Candidates (ACT/DVE/gpsimd) + folds (DVE) for nj i-tiles."""
                ck, dk = cks[ob], dks[ob]
                CU = nj * 128
                osl = slice(ob * 128, (ob + 1) * 128)
                isl = slice(ch * CHI + j0 * 128, ch * CHI + j0 * 128 + CU)
                b0 = dq.tile([128, CHI], BF16, name="b0", tag="b0")
                m1 = dq.tile([128, CHI], I16, name="m1", tag="m1")
                m2 = dq.tile([128, CHI], I16, name="m2", tag="m2")
                m3 = dq.tile([128, CHI], I16, name="m3", tag="m3")
                nc.sync.dma_start(b0[:, :CU], b0_d[osl, isl])
                nc.sync.dma_start(m1[:, :CU], m1_d[osl, isl])
                nc.sync.dma_start(m2[:, :CU], m2_d[osl, isl])
                nc.sync.dma_start(m3[:, :CU], m3_d[osl, isl])
                V = dq.tile([128, 8, CHI], BF16, name="V", tag="V")
                for k in range(8):
                    if vsplit and k >= 5:
                        nc.vector.tensor_scalar(V[:, k, :CU], b0[:, :CU],
                                                dk[:, k:k+1], ck[:, k:k+1],
                                                Alu.mult, Alu.add)
                    elif k >= 6 or (vsplit and k >= 3):
                        nc.gpsimd.tensor_scalar(V[:, k, :CU], b0[:, :CU],
                                                dk[:, k:k+1], ck[:, k:k+1],
                                                Alu.mult, Alu.add)
                    else:
                        nc.scalar.activation(V[:, k, :CU], b0[:, :CU], Act.Identity,
                                             bias=ck[:, k:k+1], scale=dk[:, k:k+1])
                nc.vector.copy_predicated(
                    V[:, 0:8:2, :CU], m1[:, :CU].unsqueeze(1).broadcast_to([128, 4, CU]),
                    V[:, 1:8:2, :CU])
                nc.vector.copy_predicated(
                    V[:, 0:8:4, :CU], m2[:, :CU].unsqueeze(1).broadcast_to([128, 2, CU]),
                    V[:, 2:8:4, :CU])
                if direct:
                    nc.vector.copy_predicated(V[:, 0, :CU], m3[:, :CU], V[:, 4, :CU])
                    _transpose_wt(V[:, 0, :CU], ch, ob, j0, nj)
                else:
                    w_t = wd.tile([128, CHI], BF16, name="wd", tag="wd")
                    nc.vector.select(w_t[:, :CU], m3[:, :CU], V[:, 4, :CU], V[:, 0, :CU])
                    wds[(ch, ob)] = w_t

            def dq_front(ch, ob, direct=False, vsplit=False):
                dq_unit(ch, ob, 0, n_cit, direct=direct, vsplit=vsplit)

            def dq_back(ch, ob):
                _transpose_wt(wds.pop((ch, ob))[:], ch, ob)

            y_sbs = {}

            def _load_half(g, ch, h):
                xt = xq.tile([128, NH, TG], BF16, name="xt", tag="xt")
                it0 = ch * n_cit + h * NH
                for q in range(2):
                    nc.sync.dma_start(xt[:, q*(NH//2):(q+1)*(NH//2), :],
                                      x_d[g][:, it0 + q*(NH//2):it0 + (q+1)*(NH//2), :])
                return xt

            def emit_pass(ch, g, obs):
                first, last = (ch == 0), (ch == n_ch - 1)
                it0 = ch * n_cit
                xts = [_load_half(g, ch, h) for h in range(2)]
                for ob in obs:
                    osl = slice(ob * 128, (ob + 1) * 128)
                    y_ps = psg.tile([128, TG], F32, name="y_ps", tag="y_ps")
                    for j in range(n_cit):
                        nc.tensor.matmul(y_ps[:], WT[:, it0 + j, osl],
                                         xts[j // NH][:, j % NH, :],
                                         start=(j == 0), stop=(j == n_cit - 1))
                    if first:
                        y_sb = ysb.tile([128, TG], BF16, name="y_sb", tag="y_sb")
                        nc.scalar.activation(y_sb[:], y_ps[:], Act.Identity,
                                             bias=biasT[:, ob:ob+1])
                        y_sbs[(g, ob)] = y_sb
                    elif not last:
                        y_sb2 = ysb.tile([128, TG], BF16, name="y_sb", tag="y_sb")
                        nc.vector.tensor_tensor(y_sb2[:], y_ps[:],
                                                y_sbs[(g, ob)][:], Alu.add)
                        y_sbs[(g, ob)] = y_sb2
                    else:
                        y_o = yout.tile([128, TG], F32, name="y_o", tag="y_o")
                        nc.vector.tensor_tensor(y_o[:], y_ps[:],
                                                y_sbs.pop((g, ob))[:], Alu.add)
                        nc.sync.dma_start(out_d[ob*128:(ob+1)*128, g*TG:(g+1)*TG],
                                          y_o[:])

            def emit_fused(g):
                y_pss = [psg.tile([128, TG], F32, name="y_ps", tag="y_ps")
                         for _ in range(n_ob)]
                for q in range(n_it // NH):       # 8 half-chunks
                    xt = _load_half(g, q // 2, q % 2)
                    for ob in range(n_ob):
                        osl = slice(ob * 128, (ob + 1) * 128)
                        for j in range(NH):
                            nc.tensor.matmul(
                                y_pss[ob][:], WT[:, q*NH + j, osl], xt[:, j, :],
                                start=(q == 0 and j == 0),
                                stop=(q == n_it // NH - 1 and j == NH - 1))
                for ob in range(n_ob):
                    y_o = yout.tile([128, TG], F32, name="y_o", tag="y_o")
                    nc.scalar.activation(y_o[:], y_pss[ob][:], Act.Identity,
                                         bias=biasT[:, ob:ob+1])
                    if g == n_g - 1:  # shorter tail: parallel half-DMAs
                        for h in range(2):
                            ts = slice(g*TG + h*(TG//2), g*TG + (h+1)*(TG//2))
                            nc.sync.dma_start(out_d[ob*128:(ob+1)*128, ts],
                                              y_o[:, h*(TG//2):(h+1)*(TG//2)])
                    else:
                        nc.sync.dma_start(out_d[ob*128:(ob+1)*128, g*TG:(g+1)*TG],
                                          y_o[:])

            # ---- schedule ----
            # Chunk 0 dequants in half-units so the first weight tiles land
            # ~13us earlier; the pass-0 matmuls for the first half start
            # before the second half's folds finish (region-tracked deps).
            for ob in range(n_ob):
                load_lut(ob)
                dq_front(0, ob, direct=True, vsplit=(ob == 0))
            nc.sync.dma_start(biasT[:], biasT_d[:])

            # Pass 0 as two ob-pair sweeps (own x tiles -> no x hostage while
            # later blocks dequant); chunk-1 fronts interleaved.
            for si, obs in enumerate(((0, 1), (2, 3))):
                for g in range(NAG):
                    if g < 2:
                        dq_front(1, si * 2 + g)
                    emit_pass(0, g, obs)

            # Passes 1-2 also run as ob-pair sweeps (x re-read once per pass)
            # so a pass never chains all 4 blocks while the last block's
            # weights are still in flight; dequant order is unchanged.
            for ch in range(1, n_ch):
                for ob in range(n_ob):
                    dq_back(ch, ob)
                if ch + 1 < n_ch:
                    for obs in ((0, 1), (2, 3)):
                        for g in range(NAG):
                            if obs == (0, 1) and g < n_ob:
                                dq_front(ch + 1, g)
                            emit_pass(ch, g, obs)
                else:
                    for g in range(NAG):
                        emit_pass(ch, g, range(n_ob))

            for g in range(NAG, n_g):
                emit_fused(g)

    nc.compile()
    return nc


def _make_xt(x2):
    # X6[g, p, a, t] = x2[g*TG+t, a*128+p] in bf16 — pure layout change
    xb = x2.astype(ml_dtypes.bfloat16)
    return np.ascontiguousarray(
        xb.reshape(T // TG, TG, IN // 128, 128).transpose(0, 3, 2, 1))


def _make_inmaps(x, qweight, lut, bias):
    x6 = _make_xt(np.asarray(x, np.float32).reshape(T, IN))
    # Bit-plane unpack (layout/dtype transform only; no lut arithmetic):
    # plane-0 bits as bf16 0/1, planes 1..3 as int16 0/1 fold predicates.
    qw = np.asarray(qweight, np.int32)[:4]
    shifts = np.arange(32, dtype=np.int32)
    bits = ((qw[:, :, :, None] >> shifts) & 1).astype(np.int16).reshape(4, OF, IN)
    b0f = bits[0].astype(ml_dtypes.bfloat16)
    in_maps = []
    for c in range(N_CORES):
        sl = slice(c * O, (c + 1) * O)
        bsl = np.asarray(bias, np.float32)[sl]
        in_maps.append({
            "x": x6,
            "b0": np.ascontiguousarray(b0f[sl]),
            "m1": np.ascontiguousarray(bits[1, sl]),
            "m2": np.ascontiguousarray(bits[2, sl]),
            "m3": np.ascontiguousarray(bits[3, sl]),
            "lut": np.ascontiguousarray(np.asarray(lut, np.float32)[sl, :]),
            "biasT": np.ascontiguousarray(bsl.reshape(O // 128, 128).T),
        })
    return in_maps


_nc_cache = None


def kernel(x, qweight, lut, bias, w_bits):
    global _nc_cache
    assert int(w_bits) == 4, f"kernel hardcodes w_bits=4, got {w_bits}"
    x = np.asarray(x, dtype=np.float32)
    assert x.shape == (B, TT, IN) and np.asarray(qweight).shape[1:] == (OF, IN // 32)

    in_maps = _make_inmaps(x, qweight, lut, bias)
    if _nc_cache is None:
        _nc_cache = _build(T, IN, O)
    res = run_bass_kernel_spmd(_nc_cache, in_maps, core_ids=list(range(N_CORES)))
    y = np.concatenate([res.results[i]["out"].T for i in range(N_CORES)], axis=1)
    return np.ascontiguousarray(y.reshape(B, TT, OF).astype(np.float32))


# revision 39
# speedup vs baseline: 1.1829x; 1.0165x over previous
"""AnyPrecisionLinear (4-bit LUT quantized linear) on 8 Trainium2 NeuronCores.

y[b,t,o] = sum_i x[b,t,i] * lut[o, idx(o,i)] + bias[o]
  idx(o,i) = 4-bit code assembled LSB-first from bit-planes qweight[0:4].

Sharding (column-parallel): qweight/lut/bias sharded along out_features into
8 shards of 512; x replicated; each core computes y[:, :, shard] and the host
concatenates along the feature axis.

Per-core kernel — transposed-output GEMM + host bit-unpack:
 - Host pre-expands the qweight bit-planes into a bf16 plane-0 tensor and
   int16 plane-1..3 masks (pure bit-unpacking / dtype layout transform; all
   lut-dependent arithmetic stays on device).  This removes the int16 AND
   chain from the DVE and makes dequant a clean 2-stage ACT->DVE pipeline:
   8 ACT affine candidates from plane 0, 3 predicated folds by planes 1..3.
 - Computes yT[o, t]: stationary = dequanted WT block [i, 128o], moving =
   x strip [i, 512 tokens]; every matmul is per-output-block AND 512-wide so
   the PE consumes each 128-channel block the moment it is dequanted.
   The host un-transposes the [O, T] result for free.
 - Schedule: NAG staged 512-token groups run one PSUM pass per dequant chunk
   (bf16 partials in SBUF); pass 0 runs as two output-block-pair sweeps with
   independent x tiles so early groups don't hold x hostage while later
   blocks dequant.  Remaining groups run fully fused (128 matmuls into 4
   resident PSUM banks, no staging).
 - bias is a per-partition ACT bias fused into the PSUM-evacuation copy.
 - x fed as bf16 (halves HBM read traffic); all DMA on HWDGE queues.
"""
import numpy as np
import ml_dtypes
from concourse import bacc, mybir, tile, masks
from concourse.bass_utils import run_bass_kernel_spmd

dt = mybir.dt
F32, BF16, I32, I16 = dt.float32, dt.bfloat16, dt.int32, dt.int16
Act = mybir.ActivationFunctionType
Alu = mybir.AluOpType

N_CORES = 8
B, TT, IN, OF = 4, 2048, 4096, 4096
T = B * TT
O = OF // N_CORES
TG = 512                         # tokens per group
# Note: DMA-XBAR weight transpose was tried and measured 1.23us per
# [128,128] block on HW (10x the cost model); PE transposes win.


def _build(T=8192, IN=4096, O=512, NAG=8):
    n_g = T // TG                # 16 token groups
    n_it = IN // 128             # 32 i-tiles
    n_ob = O // 128              # 4 output blocks
    n_ch = 4                     # dequant chunks
    CHI = IN // n_ch             # 1024 input features per chunk
    n_cit = CHI // 128           # 8 i-tiles per chunk
    NH = n_cit // 2              # 4 i-tiles per half-chunk x tile

    nc = bacc.Bacc("TRN2", target_bir_lowering=False, debug=False)
    x_d = nc.dram_tensor("x", [n_g, 128, n_it, TG], BF16, kind="ExternalInput")
    b0_d = nc.dram_tensor("b0", [O, IN], BF16, kind="ExternalInput")
    m1_d = nc.dram_tensor("m1", [O, IN], I16, kind="ExternalInput")
    m2_d = nc.dram_tensor("m2", [O, IN], I16, kind="ExternalInput")
    m3_d = nc.dram_tensor("m3", [O, IN], I16, kind="ExternalInput")
    lut_d = nc.dram_tensor("lut", [O, 16], F32, kind="ExternalInput")
    biasT_d = nc.dram_tensor("biasT", [128, n_ob], F32, kind="ExternalInput")
    out_d = nc.dram_tensor("out", [O, T], F32, kind="ExternalOutput")

    with tile.TileContext(nc) as tc:
        with tc.tile_pool(name="consts", bufs=1) as consts, \
             tc.tile_pool(name="wpool", bufs=1) as wpool, \
             tc.tile_pool(name="qpool", bufs=1) as qpool, \
             tc.tile_pool(name="dq", bufs=2) as dq, \
             tc.tile_pool(name="wd", bufs=7) as wd, \
             tc.tile_pool(name="xq", bufs=12) as xq, \
             tc.tile_pool(name="ysb", bufs=NAG * 4 + 16) as ysb, \
             tc.tile_pool(name="yout", bufs=4) as yout, \
             tc.tile_pool(name="psg", bufs=8, space="PSUM") as psg:

            ident_bf16 = consts.tile([128, 128], BF16)
            masks.make_identity(nc, ident_bf16[:])
            biasT = consts.tile([128, n_ob], F32)

            WT = wpool.tile([128, n_it, O], BF16)  # [i-part, i-tile, o]

            cks, dks = [], []

            def load_lut(ob):
                lut_t = qpool.tile([128, 16], F32, name=f"lut_t{ob}", tag=f"lut_t{ob}")
                nc.sync.dma_start(lut_t[:], lut_d[ob*128:(ob+1)*128, :])
                ck = qpool.tile([128, 8], F32, name=f"ck{ob}", tag=f"ck{ob}")
                dk = qpool.tile([128, 8], F32, name=f"dk{ob}", tag=f"dk{ob}")
                nc.vector.tensor_copy(ck[:], lut_t[:, 0:16:2])
                nc.vector.tensor_tensor(dk[:], lut_t[:, 1:16:2],
                                        lut_t[:, 0:16:2], Alu.subtract)
                cks.append(ck); dks.append(dk)

            wds = {}

            def _transpose_wt(w_ap, ch, ob, j0=0, nj=None):
                nj = n_cit if nj is None else nj
                it0 = ch * n_cit + j0
                # stage in a GEMM psum bank (same 2KB/partition footprint,
                # bitcast view) so all 8 banks serve the GEMM after dequant
                wt_f = psg.tile([128, TG], F32, name="y_ps", tag="y_ps")
                wt_ps = wt_f[:].bitcast(BF16)
                for j in range(nj):
                    nc.tensor.transpose(wt_ps[:, j*128:(j+1)*128],
                                        w_ap[:, j*128:(j+1)*128], ident_bf16[:])
                nc.scalar.copy(WT[:, it0:it0+nj, ob*128:(ob+1)*128],
                               wt_ps[:, :nj*128].rearrange("p (a b) -> p a b", a=nj))

            def dq_unit(ch, ob, j0, nj, direct=False, vsplit=False):
                """Candidates (ACT/DVE/gpsimd) + folds (DVE) for nj i-tiles."""
                ck, dk = cks[ob], dks[ob]
                CU = nj * 128
                osl = slice(ob * 128, (ob + 1) * 128)
                isl = slice(ch * CHI + j0 * 128, ch * CHI + j0 * 128 + CU)
                b0 = dq.tile([128, CHI], BF16, name="b0", tag="b0")
                m1 = dq.tile([128, CHI], I16, name="m1", tag="m1")
                m2 = dq.tile([128, CHI], I16, name="m2", tag="m2")
                m3 = dq.tile([128, CHI], I16, name="m3", tag="m3")
                nc.sync.dma_start(b0[:, :CU], b0_d[osl, isl])
                nc.sync.dma_start(m1[:, :CU], m1_d[osl, isl])
                nc.sync.dma_start(m2[:, :CU], m2_d[osl, isl])
                nc.sync.dma_start(m3[:, :CU], m3_d[osl, isl])
                V = dq.tile([128, 8, CHI], BF16, name="V", tag="V")
                for k in range(8):
                    if vsplit and k >= 5:
                        nc.vector.tensor_scalar(V[:, k, :CU], b0[:, :CU],
                                                dk[:, k:k+1], ck[:, k:k+1],
                                                Alu.mult, Alu.add)
                    elif k >= 6 or (vsplit and k >= 3):
                        nc.gpsimd.tensor_scalar(V[:, k, :CU], b0[:, :CU],
                                                dk[:, k:k+1], ck[:, k:k+1],
                                                Alu.mult, Alu.add)
                    else:
                        nc.scalar.activation(V[:, k, :CU], b0[:, :CU], Act.Identity,
                                             bias=ck[:, k:k+1], scale=dk[:, k:k+1])
                nc.vector.copy_predicated(
                    V[:, 0:8:2, :CU], m1[:, :CU].unsqueeze(1).broadcast_to([128, 4, CU]),
                    V[:, 1:8:2, :CU])
                nc.vector.copy_predicated(
                    V[:, 0:8:4, :CU], m2[:, :CU].unsqueeze(1).broadcast_to([128, 2, CU]),
                    V[:, 2:8:4, :CU])
                if direct:
                    nc.vector.copy_predicated(V[:, 0, :CU], m3[:, :CU], V[:, 4, :CU])
                    _transpose_wt(V[:, 0, :CU], ch, ob, j0, nj)
                else:
                    w_t = wd.tile([128, CHI], BF16, name="wd", tag="wd")
                    nc.vector.select(w_t[:, :CU], m3[:, :CU], V[:, 4, :CU], V[:, 0, :CU])
                    wds[(ch, ob)] = w_t

            def dq_front(ch, ob, direct=False, vsplit=False):
                dq_unit(ch, ob, 0, n_cit, direct=direct, vsplit=vsplit)

            def dq_back(ch, ob):
                _transpose_wt(wds.pop((ch, ob))[:], ch, ob)

            y_sbs = {}

            def _load_half(g, ch, h):
                xt = xq.tile([128, NH, TG], BF16, name="xt", tag="xt")
                it0 = ch * n_cit + h * NH
                for q in range(2):
                    nc.sync.dma_start(xt[:, q*(NH//2):(q+1)*(NH//2), :],
                                      x_d[g][:, it0 + q*(NH//2):it0 + (q+1)*(NH//2), :])
                return xt

            def emit_pass(ch, g, obs):
                first, last = (ch == 0), (ch == n_ch - 1)
                it0 = ch * n_cit
                xts = [_load_half(g, ch, h) for h in range(2)]
                for ob in obs:
                    osl = slice(ob * 128, (ob + 1) * 128)
                    y_ps = psg.tile([128, TG], F32, name="y_ps", tag="y_ps")
                    for j in range(n_cit):
                        nc.tensor.matmul(y_ps[:], WT[:, it0 + j, osl],
                                         xts[j // NH][:, j % NH, :],
                                         start=(j == 0), stop=(j == n_cit - 1))
                    if first:
                        y_sb = ysb.tile([128, TG], BF16, name="y_sb", tag="y_sb")
                        nc.scalar.activation(y_sb[:], y_ps[:], Act.Identity,
                                             bias=biasT[:, ob:ob+1])
                        y_sbs[(g, ob)] = y_sb
                    elif not last:
                        y_sb2 = ysb.tile([128, TG], BF16, name="y_sb", tag="y_sb")
                        nc.vector.tensor_tensor(y_sb2[:], y_ps[:],
                                                y_sbs[(g, ob)][:], Alu.add)
                        y_sbs[(g, ob)] = y_sb2
                    else:
                        y_o = yout.tile([128, TG], F32, name="y_o", tag="y_o")
                        nc.vector.tensor_tensor(y_o[:], y_ps[:],
                                                y_sbs.pop((g, ob))[:], Alu.add)
                        nc.sync.dma_start(out_d[ob*128:(ob+1)*128, g*TG:(g+1)*TG],
                                          y_o[:])

            def emit_fused(g):
                y_pss = [psg.tile([128, TG], F32, name="y_ps", tag="y_ps")
                         for _ in range(n_ob)]
                for q in range(n_it // NH):       # 8 half-chunks
                    xt = _load_half(g, q // 2, q % 2)
                    for ob in range(n_ob):
                        osl = slice(ob * 128, (ob + 1) * 128)
                        for j in range(NH):
                            nc.tensor.matmul(
                                y_pss[ob][:], WT[:, q*NH + j, osl], xt[:, j, :],
                                start=(q == 0 and j == 0),
                                stop=(q == n_it // NH - 1 and j == NH - 1))
                for ob in range(n_ob):
                    y_o = yout.tile([128, TG], F32, name="y_o", tag="y_o")
                    nc.scalar.activation(y_o[:], y_pss[ob][:], Act.Identity,
                                         bias=biasT[:, ob:ob+1])
                    if g == n_g - 1:  # shorter tail: parallel half-DMAs
                        for h in range(2):
                            ts = slice(g*TG + h*(TG//2), g*TG + (h+1)*(TG//2))
                            nc.sync.dma_start(out_d[ob*128:(ob+1)*128, ts],
                                              y_o[:, h*(TG//2):(h+1)*(TG//2)])
                    else:
                        nc.sync.dma_start(out_d[ob*128:(ob+1)*128, g*TG:(g+1)*TG],
                                          y_o[:])

            # ---- schedule ----
            # Chunk 0 dequants in half-units so the first weight tiles land
            # ~13us earlier; the pass-0 matmuls for the first half start
            # before the second half's folds finish (region-tracked deps).
            for ob in range(n_ob):
                load_lut(ob)
                dq_front(0, ob, direct=True, vsplit=(ob == 0))
            nc.sync.dma_start(biasT[:], biasT_d[:])

            # Pass 0 as two ob-pair sweeps (own x tiles -> no x hostage while
            # later blocks dequant); chunk-1 fronts interleaved.
            for si, obs in enumerate(((0, 1), (2, 3))):
                for g in range(NAG):
                    if g < 2:
                        dq_front(1, si * 2 + g)
                    emit_pass(0, g, obs)

            for ch in range(1, n_ch):
                for ob in range(n_ob):
                    dq_back(ch, ob)
                for g in range(NAG):
                    if ch + 1 < n_ch and g < n_ob:
                        dq_front(ch + 1, g)
                    emit_pass(ch, g, range(n_ob))

            for g in range(NAG, n_g):
                emit_fused(g)

    nc.compile()
    return nc


def _make_xt(x2):
    # X6[g, p, a, t] = x2[g*TG+t, a*128+p] in bf16 — pure layout change
    xb = x2.astype(ml_dtypes.bfloat16)
    return np.ascontiguousarray(
        xb.reshape(T // TG, TG, IN // 128, 128).transpose(0, 3, 2, 1))


def _make_inmaps(x, qweight, lut, bias):
    x6 = _make_xt(np.asarray(x, np.float32).reshape(T, IN))
    # Bit-plane unpack (layout/dtype transform only; no lut arithmetic):
    # plane-0 bits as bf16 0/1, planes 1..3 as int16 0/1 fold predicates.
    qw = np.asarray(qweight, np.int32)[:4]
    shifts = np.arange(32, dtype=np.int32)
    bits = ((qw[:, :, :, None] >> shifts) & 1).astype(np.int16).reshape(4, OF, IN)
    b0f = bits[0].astype(ml_dtypes.bfloat16)
    in_maps = []
    for c in range(N_CORES):
        sl = slice(c * O, (c + 1) * O)
        bsl = np.asarray(bias, np.float32)[sl]
        in_maps.append({
            "x": x6,
            "b0": np.ascontiguousarray(b0f[sl]),
            "m1": np.ascontiguousarray(bits[1, sl]),
            "m2": np.ascontiguousarray(bits[2, sl]),
            "m3": np.ascontiguousarray(bits[3, sl]),
            "lut": np.ascontiguousarray(np.asarray(lut, np.float32)[sl, :]),
            "biasT": np.ascontiguousarray(bsl.reshape(O // 128, 128).T),
        })
    return in_maps


_nc_cache = None


def kernel(x, qweight, lut, bias, w_bits):
    global _nc_cache
    assert int(w_bits) == 4, f"kernel hardcodes w_bits=4, got {w_bits}"
    x = np.asarray(x, dtype=np.float32)
    assert x.shape == (B, TT, IN) and np.asarray(qweight).shape[1:] == (OF, IN // 32)

    in_maps = _make_inmaps(x, qweight, lut, bias)
    if _nc_cache is None:
        _nc_cache = _build(T, IN, O)
    res = run_bass_kernel_spmd(_nc_cache, in_maps, core_ids=list(range(N_CORES)))
    y = np.concatenate([res.results[i]["out"].T for i in range(N_CORES)], axis=1)
    return np.ascontiguousarray(y.reshape(B, TT, OF).astype(np.float32))
